# revision 15
# baseline (speedup 1.0000x reference)
"""Additive attention (Bahdanau) Trainium2 Bass kernel.

Math (per batch b):
    q' = queries @ W_q            (Q, H)   -> stored transposed [H, Q]
    k' = keys @ W_k               (K, H)   -> stored transposed [H, K]
    scores[q,k] = sum_h w_v[h] * tanh(q'[h,q] + k'[h,k])   (+ -1e9 mask tail)
    out = softmax(scores) @ values

Distribution: work item = (batch, 128-wide key chunk, 128-wide query block);
only key chunks intersecting [0, valid_len_b) exist.  Items are dealt
round-robin onto 8 cores (identical instruction stream - pure SPMD; cores
differ only through input data).  Each item emits unnormalized flash partials
PV = exp(S) @ V and l = rowsum(exp(S)); the host sums partials over key
chunks and normalizes.  No running max is needed: |scores| <= sum|w_v| ~ 9,
so exp() cannot overflow fp32.

Device pipeline per item (processed in two 64-query halves so the
ACT-tanh / PE-matvec / ACT-exp chain pipelines tightly at the kernel's
start and end):
    PE    : q'/k' projections (bf16 operands, fp32 PSUM accumulation)
    DVE   : S_pre[h, q, k] = q'[h,q] + k'[h,k]  (bf16 broadcast add).
            q' is stored pair-duplicated [H, NQ, 2] so every operand AP has
            an innermost unit-stride pair -> DVE picks the 2x_1P bf16 mode.
    ACT   : T = tanh(S_pre) -> bf16              (one big activation / half)
    PE    : 64 accumulating bf16 matvecs with a shifted-diagonal w_v window
            -> scores PSUM tile [64q, 128k] (fp32); one extra K=1 fp32
            matmul adds the -1e9 mask row to every q row.
    ACT   : p = exp(scores)  (fp32)
    DVE   : l = rowsum(p)    (fp32)
    PE    : transpose p (fp32), then PV = p.T-tile(bf16) @ V-chunk(bf16)
            accumulated in fp32 PSUM.

bf16 is used where an operand merely streams through the PE array (fp32
moving operands stream at 1/4 rate) or where DVE's 2x bf16 mode doubles
elementwise throughput; all reductions/accumulations stay fp32.
"""

import functools
import math

import numpy as np

import concourse.bacc as bacc
import concourse.bass as bass
import concourse.tile as tile
from concourse import mybir
from concourse.bass_utils import run_bass_kernel_spmd

N_CORES = 8
B, Q, K, D, VD, H = 4, 512, 1024, 256, 256, 128
KC = 128          # keys per item
NQ = 128          # queries per item
NH = NQ // 2      # queries per processing half
NQB = Q // NQ     # q-blocks per (batch, kchunk)
NEG = -1e9

F32 = mybir.dt.float32
BF16 = mybir.dt.bfloat16
NP_BF16 = mybir.dt.np(BF16)

# Results of the last device run (for the test harness to inspect timing).
LAST_RESULTS = None


def _ensure_axon_hooks():
    """run_bass_kernel_spmd(trace=True) imports antenv.axon_hooks, which not
    every container image ships.  Provide a no-op fallback so a BASS_TRACE=1
    environment degrades to an untraced run instead of crashing."""
    try:
        import antenv.axon_hooks  # noqa: F401
    except ImportError:
        import sys
        import types

        mod = types.ModuleType("antenv.axon_hooks")
        mod.get_axon_ntff_profile_hook = lambda: None
        mod.set_axon_ntff_profile_hook = lambda h: None
        sys.modules["antenv.axon_hooks"] = mod


@functools.lru_cache(maxsize=None)
def _build_program(ni: int):
    """Build the Bass program for `ni` work items per core."""
    nc = bacc.Bacc("TRN2", target_bir_lowering=False, debug=False, num_devices=N_CORES)

    kT = nc.declare_dram_parameter("kT", [ni, D, KC], BF16, isOutput=False)
    qT = nc.declare_dram_parameter("qT", [ni, D, NQ], BF16, isOutput=False)
    vv = nc.declare_dram_parameter("vv", [ni, KC, VD], BF16, isOutput=False)
    msk = nc.declare_dram_parameter("msk", [ni, 1, KC], F32, isOutput=False)
    wq = nc.declare_dram_parameter("wq", [D, H], BF16, isOutput=False)
    wk = nc.declare_dram_parameter("wk", [D, H], BF16, isOutput=False)
    wvw = nc.declare_dram_parameter("wvw", [H, 2 * NH - 1], BF16, isOutput=False)
    ones = nc.declare_dram_parameter("ones", [1, NH], F32, isOutput=False)
    ident = nc.declare_dram_parameter("ident", [NH, NH], F32, isOutput=False)

    pv = nc.declare_dram_parameter("pv", [ni, NQ, VD], F32, isOutput=True)
    ls = nc.declare_dram_parameter("ls", [ni, NQ, 1], F32, isOutput=True)

    DT = D // 128  # d-dim tiles (2)
    add = mybir.AluOpType.add
    Tanh = mybir.ActivationFunctionType.Tanh
    Exp = mybir.ActivationFunctionType.Exp

    with tile.TileContext(nc) as tc:
        with (
            tc.tile_pool(name="consts", bufs=1) as consts,
            tc.tile_pool(name="item", bufs=3) as item,
            tc.tile_pool(name="proj", bufs=2) as proj,
            tc.tile_pool(name="spre", bufs=3) as spre_pool,
            tc.tile_pool(name="tnh", bufs=3) as tnh_pool,
            tc.tile_pool(name="small", bufs=4) as small,
            tc.tile_pool(name="psq", bufs=1, space="PSUM") as psq_pool,
            tc.tile_pool(name="psk", bufs=2, space="PSUM") as psk_pool,
            tc.tile_pool(name="pss", bufs=2, space="PSUM") as pss_pool,
            tc.tile_pool(name="pspt", bufs=1, space="PSUM") as pspt_pool,
            tc.tile_pool(name="pso", bufs=2, space="PSUM") as pso_pool,
        ):
            # Allocate constant tiles up front, but defer their DMA issue
            # until after item 0's input DMAs: the Sync HWDGE ring transfers
            # one DMA at a time (~0.65 us each), and item 0's kT/qT gate the
            # whole pipeline ramp.
            sb_wq = consts.tile([128, DT, H], BF16)
            sb_wk = consts.tile([128, DT, H], BF16)
            sb_wvw = consts.tile([H, 2 * NH - 1], BF16)
            sb_ones = consts.tile([1, NH], F32)
            sb_id = consts.tile([NH, NH], F32)

            def load_consts():
                nc.sync.dma_start(
                    out=sb_wq, in_=wq[:].rearrange("(t p) h -> p t h", p=128)
                )
                nc.sync.dma_start(
                    out=sb_wk, in_=wk[:].rearrange("(t p) h -> p t h", p=128)
                )
                nc.sync.dma_start(out=sb_wvw, in_=wvw[:])
                nc.sync.dma_start(out=sb_ones, in_=ones[:])
                nc.sync.dma_start(out=sb_id, in_=ident[:])

            for it in range(ni):
                sb_kT = item.tile([128, DT, KC], BF16, tag="kT")
                nc.sync.dma_start(
                    out=sb_kT, in_=kT[it].rearrange("(t p) k -> p t k", p=128)
                )
                sb_qT = item.tile([128, DT, NQ], BF16, tag="qT")
                nc.sync.dma_start(
                    out=sb_qT, in_=qT[it].rearrange("(t p) q -> p t q", p=128)
                )
                sb_v = item.tile([KC, VD], BF16, tag="v")
                nc.sync.dma_start(out=sb_v, in_=vv[it])
                sb_msk = item.tile([1, KC], F32, tag="msk")
                nc.sync.dma_start(out=sb_msk, in_=msk[it])

                if it == 0:
                    load_consts()

                # projections: q'^T [H, NQ] (pair-duplicated), k'^T [H, KC]
                ps_q = psq_pool.tile([H, NQ], F32)
                for t in range(DT):
                    nc.tensor.matmul(
                        ps_q, lhsT=sb_wq[:, t, :], rhs=sb_qT[:, t, :],
                        start=(t == 0), stop=(t == DT - 1),
                    )
                # qp2[h, q, j] = q'[h, q] for j in {0, 1}: the duplicated pair
                # gives the broadcast-add a unit-stride innermost dimension.
                qp2 = proj.tile([H, NQ, 2], BF16, tag="qp")
                nc.vector.tensor_copy(
                    qp2, ps_q[:].unsqueeze(2).broadcast_to((H, NQ, 2))
                )

                ps_k = psk_pool.tile([H, KC], F32)
                for t in range(DT):
                    nc.tensor.matmul(
                        ps_k, lhsT=sb_wk[:, t, :], rhs=sb_kT[:, t, :],
                        start=(t == 0), stop=(t == DT - 1),
                    )
                sb_kp = proj.tile([H, KC], BF16, tag="kp")
                nc.vector.tensor_copy(sb_kp, ps_k)

                def process_block(q0: int, nb: int):
                    """Full pipeline (add->tanh->scores->exp->PV) for queries
                    [q0, q0+nb) of the current item.  nb <= NH."""
                    qs = slice(q0, q0 + nb)
                    # S_pre[h, q, (a,b)] = q'[h, q] + k'[h, 2a+b]  (bf16, 2x)
                    spre = spre_pool.tile([H, nb, KC], BF16, tag="spre")
                    nc.vector.tensor_tensor(
                        spre[:].rearrange("h q (a b) -> h q a b", b=2),
                        sb_kp[:]
                        .rearrange("h (a b) -> h a b", b=2)
                        .unsqueeze(1)
                        .broadcast_to((H, nb, KC // 2, 2)),
                        qp2[:, qs].unsqueeze(2).broadcast_to((H, nb, KC // 2, 2)),
                        op=add,
                    )
                    tnh = tnh_pool.tile([H, nb, KC], BF16, tag="tnh")
                    nc.scalar.activation(tnh, spre, Tanh)

                    # scores[q, k] = sum_h w_v[h] * T[h, q, k]  (+ mask[k])
                    # The shifted-window slice puts w_v in lhsT column q and
                    # zeros elsewhere, so each matvec accumulates into its own
                    # PSUM row.
                    ps_s = pss_pool.tile([nb, KC], F32, tag="pss")
                    for q in range(nb):
                        nc.tensor.matmul(
                            ps_s,
                            lhsT=sb_wvw[:, NH - 1 - q: NH - 1 - q + nb],
                            rhs=tnh[:, q, :],
                            start=(q == 0), stop=False,
                        )
                    nc.tensor.matmul(
                        ps_s, lhsT=sb_ones[:, :nb], rhs=sb_msk[:],
                        start=False, stop=True,
                    )

                    p_t = small.tile([nb, KC], F32, tag="p")
                    nc.scalar.activation(p_t, ps_s, Exp)
                    l_t = small.tile([nb, 1], F32, tag="l")
                    nc.vector.reduce_sum(l_t, p_t, axis=mybir.AxisListType.X)

                    ps_pt = pspt_pool.tile([KC, nb], F32, tag="pspt")
                    nc.tensor.transpose(ps_pt, p_t, sb_id[:nb, :nb])
                    sb_pt = small.tile([KC, nb], BF16, tag="pt")
                    nc.vector.tensor_copy(sb_pt, ps_pt)

                    ps_o = pso_pool.tile([nb, VD], F32, tag="pso")
                    nc.tensor.matmul(ps_o, lhsT=sb_pt, rhs=sb_v, start=True, stop=True)
                    sb_o = small.tile([nb, VD], F32, tag="o")
                    nc.vector.tensor_copy(sb_o, ps_o)

                    nc.sync.dma_start(out=pv[it, qs], in_=sb_o)
                    nc.sync.dma_start(out=ls[it, qs], in_=l_t)

                # Smaller leading blocks shorten the pipeline ramp on the
                # first item; smaller trailing blocks shorten the drain on
                # the last one.
                if it == 0:
                    blocks = [NH // 2, NH // 2, NH]
                elif it == ni - 1:
                    blocks = [NH, NH // 2, NH // 2]
                else:
                    blocks = [NH, NH]
                q0 = 0
                for nb in blocks:
                    process_block(q0, nb)
                    q0 += nb

    if not nc.is_finalized():
        nc.finalize()
    return nc


def kernel(queries, keys, values, valid_lens, W_q, W_k, w_v):
    global LAST_RESULTS
    queries = np.ascontiguousarray(np.asarray(queries, dtype=np.float32))
    keys = np.ascontiguousarray(np.asarray(keys, dtype=np.float32))
    values = np.ascontiguousarray(np.asarray(values, dtype=np.float32))
    vl = np.asarray(valid_lens).astype(np.int64)
    W_q = np.asarray(W_q, dtype=np.float32)
    W_k = np.asarray(W_k, dtype=np.float32)
    w_v = np.asarray(w_v, dtype=np.float32)

    # ---- plan work items -------------------------------------------------
    items = []  # (b, kc, qb)
    for b in range(B):
        for kc in range(int(math.ceil(vl[b] / KC))):
            for qb in range(NQB):
                items.append((b, kc, qb))
    n_real = len(items)
    ni = (n_real + N_CORES - 1) // N_CORES
    while len(items) < ni * N_CORES:
        items.append(items[0])  # dummy duplicate, ignored at merge time

    core_items = [[items[c + N_CORES * j] for j in range(ni)] for c in range(N_CORES)]

    # ---- shared constant tensors ----------------------------------------
    wvw = np.zeros((H, 2 * NH - 1), dtype=np.float32)
    wvw[:, NH - 1] = w_v
    wvw = wvw.astype(NP_BF16)
    ones_ = np.ones((1, NH), dtype=np.float32)
    ident = np.eye(NH, dtype=np.float32)

    qTb = [np.ascontiguousarray(queries[b].T).astype(NP_BF16) for b in range(B)]
    kTb = [np.ascontiguousarray(keys[b].T).astype(NP_BF16) for b in range(B)]
    v_bf = values.astype(NP_BF16)

    in_maps = []
    for c in range(N_CORES):
        kT = np.empty((ni, D, KC), dtype=NP_BF16)
        qT = np.empty((ni, D, NQ), dtype=NP_BF16)
        vv = np.empty((ni, KC, VD), dtype=NP_BF16)
        msk = np.empty((ni, 1, KC), dtype=np.float32)
        for j, (b, kc, qb) in enumerate(core_items[c]):
            sl = slice(kc * KC, (kc + 1) * KC)
            kT[j] = kTb[b][:, sl]
            qT[j] = qTb[b][:, qb * NQ:(qb + 1) * NQ]
            vv[j] = v_bf[b, sl, :]
            msk[j, 0] = np.where(
                np.arange(kc * KC, (kc + 1) * KC) < vl[b], 0.0, NEG
            ).astype(np.float32)
        in_maps.append(
            {
                "kT": kT, "qT": qT, "vv": vv, "msk": msk,
                "wq": W_q.astype(NP_BF16), "wk": W_k.astype(NP_BF16),
                "wvw": wvw, "ones": ones_, "ident": ident,
            }
        )

    # ---- run on the 8 cores ---------------------------------------------
    _ensure_axon_hooks()
    nc = _build_program(ni)

    def run_and_merge():
        global LAST_RESULTS
        res = run_bass_kernel_spmd(nc, in_maps, list(range(N_CORES)))
        LAST_RESULTS = res
        num = np.zeros((B, Q, VD), dtype=np.float64)
        den = np.zeros((B, Q), dtype=np.float64)
        for c in range(N_CORES):
            pv = np.asarray(res.results[c]["pv"])  # [ni, NQ, VD]
            lsum = np.asarray(res.results[c]["ls"])  # [ni, NQ, 1]
            for j, (b, kc, qb) in enumerate(core_items[c]):
                if c + N_CORES * j >= n_real:
                    continue  # dummy padding item
                blk = slice(qb * NQ, (qb + 1) * NQ)
                num[b, blk] += pv[j]
                den[b, blk] += lsum[j].reshape(NQ)
        return num, den

    num, den = run_and_merge()
    # A row sum of exp(scores) is >= exp(-|w_v|_1) > 1e-6 whenever at least
    # one key is valid (valid_lens >= 1), and everything must be finite.
    # A violation means a transient device fault - retry once.
    if not (np.isfinite(num).all() and np.isfinite(den).all() and (den > 1e-30).all()):
        num, den = run_and_merge()
    return (num / den[:, :, None]).astype(np.float32)
